# revision 20
# baseline (speedup 1.0000x reference)
"""Distributed ImprovedDilatedAttention on 8 Trainium2 NeuronCores.

Problem: [2, 4096, 12, 64] q/k/v, 3 head groups with (segment, dilation) in
[(1024,1), (2048,2), (4096,4)]. Each (group, batch, segment, head) pair is an
independent dense 1024x1024 attention over head_dim 64 (m = g/r = 1024 for
every group): 56 problems total, 7 per core.

Host side packs one bf16 input block per problem, [128, 2568] = qT | kT | vp:
  qT [128, 1024] = Q^T duplicated into both partition halves (row tiling)
  kT [128, 1024] = K^T duplicated likewise (stationary operand for S^T)
  vp [128, 8, 65] = V' chunks, V' = [V | 1]; vp[j, c, :] = V'[c*128 + j]

Device, per problem (three-engine softmax pipeline):
  S^T[kj, qi] = sum_d K^T[d,kj] Q^T[d,qi]   computed in 16 units of
      (kj block 128 x qi half 512), grouped into 6 PSUM chunks of
      [128, 1536|1024] (2 slots x 3 banks); units inside a chunk alternate
      PE row strips 0-63 / 64-127 so consecutive matmuls overlap.
  E = exp(S/8): chunks alternate between ScalarE (exact ACTIVATE Exp) and
      VectorE (Schraudolph: one tensor_scalar mult+add fp32->int16 whose
      int16 bits ARE bf16(2^z); ~3% PWL err on half the cells, which the
      softmax normalization mostly cancels -- validated 1.6e-2 end-to-end).
      The (kj block, qi half) -> engine map is chosen so every query sees
      exactly 4 approximated key blocks (balanced, minimizes worst case).
  out[m, qi] = sum_kj V'[kj, m] E[kj, qi]   V' is the stationary operand
      (8 cheap 65-col LDWEIGHTS per problem instead of 64 128-col ones,
      which were the old Tensor-queue bottleneck), E moving at N=512,
      accumulated over the 8 kj chunks into a persistent [65, 1024] PSUM
      tile (2 banks). Evacuation is split: ScalarE copies one qi half,
      VectorE the other; PV matmuls of problem p-1 interleave with the
      S chunks of problem p to keep the PE busy while exps drain.
out[0:64, qi] is the unnormalized O^T, row 64 is sumexp. Host divides and
scatters into the dilated positions (zeros elsewhere).
"""

import numpy as np

B, N, H, D = 2, 4096, 12, 64
SEG = [1024, 2048, 4096]
DIL = [1, 2, 4]
NGROUPS = 3
HPG = H // NGROUPS  # 4 heads per group
M = 1024            # dilated tokens per segment (g // r, same for all groups)
NPROB = 56
NCORES = 8
PPC = NPROB // NCORES  # 7 problems per core

_CACHE = {}

# Schraudolph exp on VectorE: bits_i16 = trunc(S * EXP_A + EXP_B); the int16
# bit pattern equals bf16(exp(S/8)) under a piecewise-linear 2^f approx.
# EXP_A = 16*log2(e) (the /8 score scale folded in); EXP_B tuned numerically
# (127<<7 minus ~5.1 PWL-centering correction, assuming truncating convert).
EXP_A = float(np.float32(16.0 * np.log2(np.e)))
EXP_B = 16250.875

# unit = (kj block j, qi half qh); SEQ is the emission order. Chunks are the
# 8 consecutive pairs; even chunks -> ScalarE exact exp, odd chunks ->
# VectorE Schraudolph. DVE cells are {(odd j, 0)} + {(even j, 1)}: each qi
# half sees exactly 4 approximated kj blocks. Units inside a chunk alternate
# row strips (j parity) so their matmuls overlap on the PE.
# qh0 cells fill chunks 0-3, qh1 cells chunks 4-7: PSUM accumulator bank 0
# (qi half 0) completes after chunk 3's PV quad and can be evacuated two
# groups before bank 1 - so the next problem's accumulation never waits.
SEQ = [(0, 0), (2, 0),   # c0 ACT
       (1, 0), (3, 0),   # c1 DVE
       (4, 0), (6, 0),   # c2 ACT
       (5, 0), (7, 0),   # c3 DVE
       (1, 1), (3, 1),   # c4 ACT
       (0, 1), (2, 1),   # c5 DVE
       (5, 1), (7, 1),   # c6 ACT
       (4, 1), (6, 1)]   # c7 DVE
NCHUNK = 8
POS = {cell: i for i, cell in enumerate(SEQ)}  # unit -> eS slot index


def _bf16():
    import ml_dtypes

    return ml_dtypes.bfloat16


def _groups():
    for i, (g, r) in enumerate(zip(SEG, DIL)):
        yield i, g, r, i % r, N // g


def _pack(query, key, value):
    """-> per-problem input blocks [56, 128, 2568] = qT | kT | vp (bf16)."""
    bf16 = _bf16()
    qs, ks, vs = [], [], []
    for i, g, r, off, s in _groups():
        idx = off + r * np.arange(g // r)
        hsl = slice(i * HPG, (i + 1) * HPG)

        def grab(x):
            return x.reshape(B, s, g, H, D)[:, :, idx][:, :, :, hsl, :]

        qg = grab(query)  # [B, s, m, hpg, D]
        kg = grab(key)
        vg = grab(value)
        qT = np.ascontiguousarray(qg.transpose(0, 1, 3, 4, 2)).reshape(-1, D, M)
        kT = np.ascontiguousarray(kg.transpose(0, 1, 3, 4, 2)).reshape(-1, D, M)
        # duplicate into both partition halves for 2-way PE row tiling
        qs.append(np.concatenate([qT, qT], axis=1))  # [n, 128, M]
        ks.append(np.concatenate([kT, kT], axis=1))
        v65 = np.concatenate(
            [vg, np.ones((*vg.shape[:-1], 1), np.float32)], axis=-1
        )  # [B, s, m, hpg, 65]
        vp = np.ascontiguousarray(v65.transpose(0, 1, 3, 2, 4)).reshape(-1, M, 65)
        vp = np.ascontiguousarray(vp.reshape(-1, 8, 128, 65).transpose(0, 2, 1, 3))
        vs.append(vp)
    qTp = np.concatenate(qs).astype(bf16)   # [56, 128, 1024]
    kTp = np.concatenate(ks).astype(bf16)   # [56, 128, 1024]
    vpp = np.concatenate(vs).astype(bf16)   # [56, 128, 8, 65]
    return np.concatenate(
        [qTp, kTp, vpp.reshape(NPROB, 128, 520)], axis=2
    )  # [56, 128, 2568]


def _unpack(outT):
    """outT [56, 65, 1024] (m-row, qi-col) -> full output."""
    o = outT.transpose(0, 2, 1)  # [56, qi, 65]
    o = o[:, :, :64] / o[:, :, 64:65]  # [56, qi, 64]
    out = np.zeros((B, N, H, D), np.float32)
    ofs = 0
    for i, g, r, off, s in _groups():
        idx = off + r * np.arange(g // r)
        n_i = B * s * HPG
        og = o[ofs : ofs + n_i].reshape(B, s, HPG, M, D).transpose(0, 1, 3, 2, 4)
        out.reshape(B, s, g, H, D)[:, :, idx, i * HPG : (i + 1) * HPG, :] = og
        ofs += n_i
    return out


def _build(for_hw=True):
    import concourse.bacc as bacc
    import concourse.bass as bass
    import concourse.mybir as mybir
    import concourse.tile as tile

    f32 = mybir.dt.float32
    i16 = mybir.dt.int16
    bf = mybir.dt.bfloat16
    nc = bacc.Bacc("TRN2", target_bir_lowering=False, debug=False,
                   enable_asserts=False)
    inx = nc.dram_tensor("inx", [PPC, 128, 2568], bf, kind="ExternalInput").ap()
    outT = nc.dram_tensor("outT", [PPC, 65, 1024], f32, kind="ExternalOutput").ap()

    with tile.TileContext(nc) as tc:
        with (
            tc.tile_pool(name="inp", bufs=3) as inp,
            tc.tile_pool(name="exps", bufs=2) as exps,
            tc.tile_pool(name="outp", bufs=2) as outp,
            tc.tile_pool(name="spool", bufs=3, space=bass.MemorySpace.PSUM) as spool,
            tc.tile_pool(name="pvpool", bufs=1, space=bass.MemorySpace.PSUM) as pvp,
        ):
            its = {}

            def load_input(p):
                it = inp.tile([128, 2568], bf, tag="it")
                # piece 1 = qT + kT blocks 0-3 (covers chunks c0/c1), piece 2
                # = the rest; lets the first S matmuls start one DMA earlier
                nc.sync.dma_start(out=it[:, 0:1536], in_=inx[p][:, 0:1536])
                nc.sync.dma_start(out=it[:, 1536:2568], in_=inx[p][:, 1536:2568])
                its[p] = it

            def emit_pv_chunk(pvt, vpt, eS, c):
                # PV matmuls for the 2 cells exp'd by chunk c: V' stationary,
                # E moving, accumulated into the persistent [65, 1024] tile
                # (qi halves live in separate PSUM banks). start/stop when the
                # cell is its qi half's first/last in SEQ order.
                for u in (2 * c, 2 * c + 1):
                    c8, qh = SEQ[u]
                    nc.tensor.matmul(
                        pvt[:, qh * 512 : (qh + 1) * 512],
                        vpt[:, c8, :],
                        eS[:, u * 512 : (u + 1) * 512],
                        start=(u % 8 == 0), stop=(u % 8 == 7),
                    )

            def flush_pv(pend):
                # emit the PV quad for the oldest pending chunk pair; each
                # accumulator bank is evacuated right after its last quad
                # (bank 0 after g1, bank 1 after g3 - staggered, so the next
                # problem's accumulation never WAR-waits on a copy)
                st, g = pend.popleft()
                emit_pv_chunk(st["pvt"], st["vpt"], st["eS"], 2 * g)
                emit_pv_chunk(st["pvt"], st["vpt"], st["eS"], 2 * g + 1)
                if g == 1:
                    st["ot"] = outp.tile([65, 1024], f32, tag="ot", name="ot")
                    nc.scalar.copy(out=st["ot"][:, 0:512], in_=st["pvt"][:, 0:512])
                elif g == 3:
                    nc.vector.tensor_copy(
                        out=st["ot"][:, 512:1024], in_=st["pvt"][:, 512:1024]
                    )
                    nc.sync.dma_start(out=outT[st["p"]], in_=st["ot"])

            from collections import deque

            pend = deque()
            load_input(0)
            for p in range(PPC):
                if p + 1 < PPC:
                    load_input(p + 1)
                it = its.pop(p)
                qt = it[:, 0:1024]
                kt = it[:, 1024:2048]
                vpt = it[:, 2048:2568].rearrange("p (c m) -> p c m", m=65)
                eS = exps.tile([128, 8192], bf, tag="eS")
                pvt = pvp.tile([65, 1024], f32, tag="pv")
                st = {"pvt": pvt, "eS": eS, "vpt": vpt, "p": p}
                # Chunk pairs: 4 S matmuls with alternating row strips (odd
                # chunk emitted cell-reversed so strips go s0,s1,s0,s1), both
                # exps, then the PV quad from TWO pairs back - far enough that
                # its exps and the accumulator copies are never on the PE's
                # critical path.
                for g in range(4):
                    c0, c1 = 2 * g, 2 * g + 1
                    sch_a = spool.tile([128, 1024], f32, tag="s")
                    sch_b = spool.tile([128, 1024], f32, tag="s")
                    schs = [sch_a, sch_b]
                    # interleave the ACT chunk's (even-j, strip 0) and DVE
                    # chunk's (odd-j, strip 1) matmuls so row strips alternate
                    for u in range(2):
                        for ci, c in enumerate((c0, c1)):
                            j, qh = SEQ[2 * c + u]
                            st64 = (j % 2) * 64
                            nc.tensor.matmul(
                                schs[ci][:, u * 512 : (u + 1) * 512],
                                kt[st64 : st64 + 64, j * 128 : (j + 1) * 128],
                                qt[st64 : st64 + 64, qh * 512 : (qh + 1) * 512],
                                start=True, stop=True,
                                tile_position=(st64, 0),
                            )
                    for ci, c in enumerate((c0, c1)):
                        esl = eS[:, 2 * c * 512 : (2 * c + 2) * 512]
                        if c % 2 == 0:
                            nc.scalar.activation(
                                esl, schs[ci],
                                mybir.ActivationFunctionType.Exp, scale=0.125,
                            )
                        else:
                            nc.vector.tensor_scalar(
                                out=esl.bitcast(i16), in0=schs[ci],
                                scalar1=EXP_A, scalar2=EXP_B,
                                op0=mybir.AluOpType.mult, op1=mybir.AluOpType.add,
                            )
                    pend.append((st, g))
                    if len(pend) > 2:
                        # two quads back-to-back: longer PV streaks halve the
                        # S<->PV stream transitions on the PE
                        flush_pv(pend)
                        flush_pv(pend)
            while pend:
                flush_pv(pend)

    nc.compile()
    if for_hw:
        from concourse.bass_interp import get_hw_module

        nc.m = get_hw_module(nc.m)
    return nc


def _numpy_fallback(query, key, value, causal):
    out = np.zeros((B, N, H, D), np.float32)
    for i, g, r, off, s in _groups():
        idx = off + r * np.arange(g // r)
        hsl = slice(i * HPG, (i + 1) * HPG)
        q = query.reshape(B, s, g, H, D)[:, :, idx][:, :, :, hsl, :]
        k = key.reshape(B, s, g, H, D)[:, :, idx][:, :, :, hsl, :]
        v = value.reshape(B, s, g, H, D)[:, :, idx][:, :, :, hsl, :]
        scores = np.einsum("bsqhd,bskhd->bshqk", q, k) / np.sqrt(D).astype(np.float32)
        if causal:
            mask = np.tril(np.ones((g // r, g // r), dtype=bool))
            scores = np.where(mask, scores, np.float32(np.finfo(np.float32).min))
        scores -= scores.max(axis=-1, keepdims=True)
        p = np.exp(scores)
        p /= p.sum(axis=-1, keepdims=True)
        o = np.einsum("bshqk,bskhd->bsqhd", p, v)
        out.reshape(B, s, g, H, D)[:, :, idx, hsl, :] = o
    return out


def _in_maps(query, key, value):
    inx = _pack(query, key, value)
    return [
        {"inx": np.ascontiguousarray(inx[k * PPC : (k + 1) * PPC])}
        for k in range(NCORES)
    ]


def kernel(query, key, value, is_causal):
    query = np.asarray(query, dtype=np.float32)
    key = np.asarray(key, dtype=np.float32)
    value = np.asarray(value, dtype=np.float32)
    causal = bool(np.asarray(is_causal).item()) if np.ndim(is_causal) == 0 else bool(
        is_causal
    )
    if causal:
        return _numpy_fallback(query, key, value, causal)

    from concourse import bass_utils

    if "nc" not in _CACHE:
        _CACHE["nc"] = _build()
    nc = _CACHE["nc"]

    res = bass_utils.run_bass_kernel_spmd(
        nc, _in_maps(query, key, value), core_ids=list(range(NCORES))
    )
    outT = np.concatenate([res.results[k]["outT"] for k in range(NCORES)])
    return _unpack(outT)
